# revision 17
# baseline (speedup 1.0000x reference)
"""DenseCL contrastive-logits kernel for 8 Trainium2 NeuronCores.

Contract: kernel(**inputs) takes the FULL unsharded inputs (named as in
setup_inputs) and returns the full [32, 65537, 50] float32 output.

Sharding:
  * The 65536-wide negative queues are split along the queue axis across
    the 8 cores (8192 columns each).
  * The match/gather stage (cosine + argmax + d_q gather) runs with 4x
    redundancy: each core matches the 16 batches of its half (cores 0-3:
    batches 0-15, cores 4-7: 16-31, reordered so the core's own 4 batches
    come first).  That makes HALF of each core's out_d shard computable
    BEFORE any cross-core exchange, which matters because the collective
    path has a ~50-70 us fixed latency floor (CC-core init + cross-device
    rank-sync barrier) that is independent of when we trigger it.  The
    other half's d_qm arrives via a 2-rank AllGather (200 KB) between
    partner cores c <-> c+4 and is written in a second pass.
  * The remote half is recovered uniformly (same program on all cores) as
    block0 + block1 - local in fp32 DVE math, which is exact for fp16
    inputs.

Precision: the match cosine runs with fp16 inputs / fp32 PSUM accumulate.
Validated on the fixed seed-0 inputs: the fp16-input argmax matches the
fp32 reference exactly, with a top-2 margin of 5.8e-3 -- three orders
above PE accumulation-order noise.  The big negative-logit matmuls and
outputs are fp16 (~4e-4 relative error).  fp16 subnormals are flushed on
the host (the PE weight path mishandles them).

Math (per batch b, t = 1/tau = 5 folded into the one-hot):
  cosT[j, i] = sum_c feat_q[b, c, j] * feat_k[b, c, i]     (PE fp16, 2 batches
                                                            packed via col tiling)
  onehotT[j, i] = t * (cosT[j, i] >= max_i cosT[j, :])      (DVE)
  onehot = onehotT^T                                        (PE transpose)
  d_qm5[d, j] = sum_i d_qT[b, i, d] * onehot[i, j]          (PE fp16)
  out_d[q, b, s] = sum_d queue_d[d, q] * d_qm5[b, d, s]     (PE fp16, q-shard)
  out_g[b, q]   = sum_d t * g_q[b, d] * queue_g[d, q]       (PE fp16, q-shard)
  pos_d[b, s]   = sum_d d_k[b, d, s] * d_qm5[b, d, s]       (fp32, local b)
  pos_g[b]      = t * sum_d g_q[b, d] * g_k[b, d]           (fp32, local b)
"""

import numpy as np

BS, DIM, S, CF, Q = 32, 128, 49, 2048, 65536
NCORES = 8
QS = Q // NCORES          # 8192 queue columns per core
BL = BS // NCORES         # 4 batches owned per core (pos output)
HB = 16                   # batches matched locally (the core's half)
CT = CF // 128            # 16 contraction chunks for the cosine
QT = QS // 128            # 64 queue tiles per core
HCOL = HB * S             # 784 columns per half in the big matmul
GW = 8 * S                # 392 columns per matmul group (8 batches)
INV_TAU = 5.0

_CACHE = {}


def _install_tile_drain_patch():
    """walrus in this container rejects instructions with >1 sync wait
    ("Too many sync wait commands" in setupSyncWait).  TileContext's
    end-of-kernel drain carries one wait per semaphore used; split them
    across a chain of single-wait drain instructions (same engine, same
    semantics)."""
    import concourse.tile as tile_mod
    import concourse.mybir as mybir
    from concourse.vector_clock import ScopedClock

    if getattr(tile_mod.TileContext, "_drain_patch_installed", False):
        return

    def _drain_and_barrier(self, tick_clock, wait_clock):
        nc = self.nc
        drain_inst = nc.sync.drain()
        wait_clock.add_sem_waits(
            drain_inst.ins, ScopedClock({None: tick_clock.global_clock})
        )
        waits = list(drain_inst.ins.sync_info.on_wait)
        if len(waits) > 1:
            drain_inst.ins.sync_info = mybir.SyncInfo(
                on_wait=waits[:1], on_update=[]
            )
            for i in range(1, len(waits)):
                extra = nc.sync.drain()
                extra.ins.sync_info = mybir.SyncInfo(
                    on_wait=waits[i : i + 1], on_update=[]
                )
        nc.all_engine_barrier()
        assert self.sems is not None
        popped = nc._tile_sem_poison_stack.pop()
        assert popped is self._sem_poison
        nc.clear_and_free_semaphores(list(self.sems.allocated().values()))
        nc.all_engine_barrier()

    tile_mod.TileContext._drain_and_barrier = _drain_and_barrier
    tile_mod.TileContext._drain_patch_installed = True


def _split_multi_waits(nc, mybir, limit=1):
    """walrus codegen here rejects instructions with more than one sync
    wait.  Hoist excess waits onto InstNoOp carriers inserted immediately
    before the offender in the same block (same engine stream => same
    semantics: all waits still execute before the instruction)."""
    n_new = 0
    for f in nc.m.functions:
        for bb in f.blocks:
            new_list = []
            changed = False
            for inst in bb.instructions:
                si = inst.sync_info
                waits = list(si.on_wait) if si is not None else []
                if len(waits) > limit:
                    for w in waits[limit:]:
                        n_new += 1
                        nop = mybir.InstNoOp(name=f"WS-{n_new}")
                        nop.engine = inst.engine
                        nop.sync_info = mybir.SyncInfo(
                            on_wait=[w], on_update=[]
                        )
                        new_list.append(nop)
                    inst.sync_info = mybir.SyncInfo(
                        on_wait=waits[:limit], on_update=list(si.on_update)
                    )
                    changed = True
                new_list.append(inst)
            if changed:
                bb.instructions = new_list


def _build():
    if "nc" in _CACHE:
        return _CACHE["nc"]

    _install_tile_drain_patch()

    import concourse.bass as bass
    import concourse.mybir as mybir
    from concourse.tile import TileContext
    from concourse.masks import make_identity

    f32 = mybir.dt.float32
    f16 = mybir.dt.float16
    X = mybir.AxisListType.X

    nc = bass.Bass()

    # ---- DRAM I/O (per-core slices prepared on the host, SBUF layouts).
    # Batch order inside fqL/fkL/d_qTL: the core's own 4 batches first,
    # then the remaining 12 of its half. ----
    fqL = nc.dram_tensor("fqL", [128, CT, HB, S], f16, kind="ExternalInput")
    fkL = nc.dram_tensor("fkL", [128, CT, HB, S], f16, kind="ExternalInput")
    d_qTL = nc.dram_tensor("d_qTL", [S, HB, DIM], f16, kind="ExternalInput")
    d_kL = nc.dram_tensor("d_kL", [DIM, BL, S], f32, kind="ExternalInput")
    g_qL = nc.dram_tensor("g_qL", [BL, DIM], f32, kind="ExternalInput")
    g_kL = nc.dram_tensor("g_kL", [BL, DIM], f32, kind="ExternalInput")
    g_qT5 = nc.dram_tensor("g_qT5", [DIM, BS], f16, kind="ExternalInput")
    qg = nc.dram_tensor("qg", [DIM, QS], f16, kind="ExternalInput")
    qd = nc.dram_tensor("qd", [DIM, QS], f16, kind="ExternalInput")

    # out_d is split into one tensor per half; batch axis in KERNEL order
    # (host permutes back in assemble()).  Their q rows are written two per
    # SBUF partition (even/odd interleave via the host-side qd layout) so
    # each DMA descriptor covers 3136 contiguous bytes.
    out_dA = nc.dram_tensor("out_dA", [QS, HB, S], f16, kind="ExternalOutput")
    out_dB = nc.dram_tensor("out_dB", [QS, HB, S], f16, kind="ExternalOutput")
    out_g = nc.dram_tensor("out_g", [BS, QS], f16, kind="ExternalOutput")
    out_pos = nc.dram_tensor("out_pos", [BL, 1 + S], f32, kind="ExternalOutput")

    with TileContext(nc) as tc:
        with (
            tc.tile_pool(name="const", bufs=1) as const_pool,
            tc.tile_pool(name="queues", bufs=1) as queue_pool,
            tc.tile_pool(name="feat", bufs=1) as feat_pool,
            tc.tile_pool(name="dqm", bufs=1) as dqm_pool,
            tc.tile_pool(name="small", bufs=3) as small_pool,
            tc.tile_pool(name="stage", bufs=8) as stage_pool,
            tc.tile_pool(name="dram", bufs=1, space="DRAM") as dram_pool,
        ):
            # ---- static loads; feat on the sync queue, the rest on the
            # scalar queue so the ~0.6us per-DMA issue costs overlap ----
            fq_sb = feat_pool.tile([128, CT, HB, S], f16, tag="fq")
            fk_sb = feat_pool.tile([128, CT, HB, S], f16, tag="fk")
            for q4 in range(4):
                sl = slice(q4 * 4, (q4 + 1) * 4)
                # alternate queues so fq/fk quarters stream in parallel
                eng_a = nc.sync if q4 % 2 == 0 else nc.scalar
                eng_b = nc.scalar if q4 % 2 == 0 else nc.sync
                eng_a.dma_start(fq_sb[:, sl], fqL[:, sl])
                eng_b.dma_start(fk_sb[:, sl], fkL[:, sl])

            d_qT_sb = const_pool.tile([128, HB, DIM], f16)   # padded K
            nc.vector.memset(d_qT_sb[:], 0.0)
            nc.scalar.dma_start(d_qT_sb[:S, :, :], d_qTL[:, :, :])
            g_qT5_sb = const_pool.tile([128, BS], f16)
            nc.scalar.dma_start(g_qT5_sb[:], g_qT5[:, :])
            qd_sb = queue_pool.tile([128, QS], f16, tag="qd")
            nc.scalar.dma_start(qd_sb[:], qd[:, :])
            qg_sb = queue_pool.tile([128, QS], f16, tag="qg")
            nc.scalar.dma_start(qg_sb[:], qg[:, :])
            d_k_sb = const_pool.tile([128, BL, S], f32)
            nc.scalar.dma_start(d_k_sb[:], d_kL[:, :, :])
            g_q_sb = const_pool.tile([BL, DIM], f32)
            nc.scalar.dma_start(g_q_sb[:], g_qL[:, :])
            g_k_sb = const_pool.tile([BL, DIM], f32)
            nc.scalar.dma_start(g_k_sb[:], g_kL[:, :])

            # ---- constants ----
            ident = const_pool.tile([128, 128], f32)
            make_identity(nc, ident)
            ident16 = const_pool.tile([128, 128], f16)
            nc.vector.tensor_copy(ident16[:], ident[:])
            ones = const_pool.tile([128, 1], f32)
            nc.vector.memset(ones, 1.0)
            oneh = const_pool.tile([128, HB, S], f16)   # zero-padded K rows
            nc.vector.memset(oneh[:], 0.0)

            posd_sb = const_pool.tile([S, BL], f32)          # local pos_d [s, b]
            pos_sb = const_pool.tile([BL, 1 + S], f32)

            # ---- phase 1: match + gather for the 16 half batches, in two
            # waves of 8 so four 1-bank PSUM accumulators suffice ----
            dqm_loc = dqm_pool.tile([128, HCOL], f16, tag="dqml")
            p1_psum = tc.tile_pool(name="p1psum", bufs=1, space="PSUM")
            pcos_pool = p1_psum.__enter__()
            p1b_psum = tc.tile_pool(name="p1bpsum", bufs=2, space="PSUM")
            paux_pool = p1b_psum.__enter__()
            with nc.named_scope("p1"):
                # 4 waves of 4 batches: wave w's argmax/gather tail (whose
                # chains pipeline through double-buffered poh/pdqm) hides
                # under wave w+1's cosine accumulation.
                pcos_t = [
                    pcos_pool.tile([128, S], f32, tag=f"pcos{p}", name=f"pcos{p}")
                    for p in range(2)
                ]
                for w in range(4):
                    for t in range(CT):
                        for pi in range(2):
                            for hf in range(2):
                                bi = w * 4 + 2 * pi + hf
                                nc.tensor.matmul(
                                    pcos_t[pi][64 * hf : 64 * hf + S, :],
                                    fq_sb[:, t, bi, :],
                                    fk_sb[:, t, bi, :],
                                    start=(t == 0),
                                    stop=(t == CT - 1),
                                    tile_position=(0, 64 * hf),
                                    skip_group_check=True,
                                )
                    for pi in range(2):
                        for hf in range(2):
                            bi = w * 4 + 2 * pi + hf
                            csl = pcos_t[pi][64 * hf : 64 * hf + S, :]
                            cmax = small_pool.tile([S, 1], f32, tag="cmax")
                            nc.vector.reduce_max(out=cmax[:], in_=csl, axis=X)
                            onehT = small_pool.tile([S, S], f16, tag="onehT")
                            nc.vector.tensor_scalar(
                                onehT[:], csl, cmax[:], INV_TAU,
                                mybir.AluOpType.is_ge, mybir.AluOpType.mult,
                            )
                            poh = paux_pool.tile([S, S], f16, tag="poh")
                            nc.tensor.transpose(poh, onehT[:], ident16[:S, :S])
                            nc.scalar.copy(oneh[:S, bi, :], poh[:])
                            pdqm = paux_pool.tile([128, S], f32, tag="pdqm")
                            nc.tensor.matmul(
                                pdqm, d_qT_sb[:, bi, :], oneh[:, bi, :],
                                start=True, stop=True,
                            )
                            nc.scalar.copy(
                                dqm_loc[:, bi * S : (bi + 1) * S], pdqm[:]
                            )

            # ---- 2-rank AllGather with the partner core (c <-> c+4):
            # exchange the 16-batch d_qm5 blocks (200 KB each).  SWDGE
            # staging so the trigger isn't held by unrelated HWDGE-lane
            # completions. ----
            with nc.named_scope("gather"):
                ag_in = dram_pool.tile([128, HCOL], f16, name="ag_in")
                ag_out = dram_pool.tile([2 * 128, HCOL], f16, name="ag_out")
                nc.gpsimd.dma_start(ag_in[:], dqm_loc[:])
                nc.gpsimd.collective_compute(
                    "AllGather",
                    mybir.AluOpType.bypass,
                    replica_groups=[[r, r + 4] for r in range(4)],
                    ins=[ag_in[:].opt()],
                    outs=[ag_out[:].opt()],
                )

            # ---- window work: pos + out_g (independent of the collective) ----
            with nc.named_scope("pos"):
                for bi in range(BL):
                    # pos_d[bi, :] = ones.T @ (d_k * d_qm5); own 4 batches
                    # sit at kernel positions 0..3
                    prod = small_pool.tile([128, S], f32, tag="prod")
                    nc.vector.tensor_tensor(
                        prod[:],
                        d_k_sb[:, bi, :],
                        dqm_loc[:, bi * S : (bi + 1) * S],
                        mybir.AluOpType.mult,
                    )
                    ppos = pcos_pool.tile([S, 1], f32, tag="ppos")
                    nc.tensor.matmul(
                        ppos, prod[:], ones[:], start=True, stop=True
                    )
                    nc.vector.tensor_copy(posd_sb[:, bi : bi + 1], ppos[:])
                prodg = small_pool.tile([BL, DIM], f32, tag="prodg")
                nc.vector.tensor_tensor(
                    prodg[:], g_q_sb[:], g_k_sb[:], mybir.AluOpType.mult
                )
                posg = small_pool.tile([BL, 1], f32, tag="posg")
                nc.vector.reduce_sum(out=posg[:], in_=prodg[:], axis=X)
                nc.vector.tensor_scalar_mul(pos_sb[:, 0:1], posg[:], INV_TAU)
                pposT = pcos_pool.tile([BL, S], f32, tag="pposT")
                nc.tensor.transpose(pposT, posd_sb[:], ident[:S, :S])
                nc.vector.tensor_copy(pos_sb[:, 1:], pposT[:])
                nc.sync.dma_start(out_pos[:, :], pos_sb[:])

            p1b_psum.__exit__(None, None, None)
            p1_psum.__exit__(None, None, None)
            pg_ctx = tc.tile_pool(name="pg", bufs=2, space="PSUM")
            pg_pool = pg_ctx.__enter__()

            # ---- out_g: 4 q-tiles col-packed per PSUM bank so the fp32
            # PSUM -> fp16 SBUF cast runs 128 partitions wide ----
            with nc.named_scope("gphase"):
                gst = stage_pool.tile([128, 4, 512], f16, tag="gstage")
                for k in range(4):
                    pg = pg_pool.tile([128, 512], f32, tag="pg")
                    for a in range(4):
                        nt = k * 4 + a
                        nc.tensor.matmul(
                            pg[32 * a : 32 * (a + 1), :],
                            g_qT5_sb[:],
                            qg_sb[:, nt * 512 : (nt + 1) * 512],
                            start=True,
                            stop=True,
                            tile_position=(0, 32 * a),
                            skip_group_check=True,
                        )
                    nc.vector.tensor_copy(gst[:, k, :], pg[:])
                # out_g[b, (k*4 + a)*512 + n] <- gst[32a + b, k, n]
                ogv = out_g.rearrange("b (k a n) -> a b k n", k=4, a=4, n=512)
                for a in range(4):
                    nc.sync.dma_start(
                        ogv[a], gst[32 * a : 32 * (a + 1), :, :]
                    )

            pg_ctx.__exit__(None, None, None)
            pmm_ctx = tc.tile_pool(name="pmm", bufs=4, space="PSUM")
            pmm_pool = pmm_ctx.__enter__()

            def half_pass(tag, dqm_src, out_t):
                """out_t[q, :, :] = qd^T @ dqm_src over the q shard, 256
                q rows per iteration.  The host interleaved qd columns so
                partition p of the even/odd matmul pair holds q rows 2p
                and 2p+1 of the block: each DMA then writes 3136
                contiguous bytes per partition (two adjacent q rows)."""
                ov = out_t.rearrange("(qb p h) b s -> p qb h b s", p=128, h=2)
                for qb in range(QS // 256):
                    stg = stage_pool.tile([128, 2, HB, S], f16, tag=tag)
                    for e in range(2):
                        pmm = pmm_pool.tile([128, 2, 512], f32, tag="pmm")
                        for h in range(2):
                            nc.tensor.matmul(
                                pmm[:, h, :GW],
                                qd_sb[:, (2 * qb + e) * 128 : (2 * qb + e + 1) * 128],
                                dqm_src[:, h * GW : (h + 1) * GW],
                                start=True,
                                stop=True,
                                skip_group_check=True,
                            )
                        src = pmm[:, :, :GW].rearrange(
                            "p h (b s) -> p h b s", b=8
                        )
                        dst = stg[:, e].rearrange("p (h b) s -> p h b s", h=2)
                        if e == 0:
                            nc.vector.tensor_copy(dst, src)
                        else:
                            nc.scalar.copy(dst, src)
                    if qb % 2 == 0:
                        nc.sync.dma_start(ov[:, qb], stg[:])
                    else:
                        nc.scalar.dma_start(ov[:, qb], stg[:])

            # ---- pass A: the local half's columns, before the collective ----
            with nc.named_scope("passA"):
                half_pass("stageA", dqm_loc, out_dA)

            # Scheduler fence: everything above must be scheduled before
            # the collective-gated loads below.
            tc.no_sync_barrier()

            # ---- unpack + reconstruct the remote half:
            # remote = block0 + block1 - local, exact in fp32 for fp16 data ----
            with nc.named_scope("gather2"):
                ago = ag_out[:].rearrange("(c p) s -> p c s", c=2)
                blk = dqm_pool.tile([128, 2, HCOL], f16, tag="blk")
                nc.gpsimd.dma_start(blk[:], ago[:, :, :])
                dqm_rem = dqm_pool.tile([128, HCOL], f16, tag="dqmr")
                bsum = dqm_pool.tile([128, HCOL], f32, tag="bsum")
                nc.gpsimd.tensor_tensor(
                    bsum[:], blk[:, 0, :], blk[:, 1, :], mybir.AluOpType.add
                )
                nc.gpsimd.tensor_tensor(
                    dqm_rem[:], bsum[:], dqm_loc[:], mybir.AluOpType.subtract
                )

            # ---- pass B: the remote half's columns ----
            with nc.named_scope("passB"):
                half_pass("stageB", dqm_rem, out_dB)
            pmm_ctx.__exit__(None, None, None)

    _split_multi_waits(nc, mybir)

    _CACHE["nc"] = nc
    return nc


def _local_order(c):
    """Kernel-side batch order for core c: its own 4 batches, then the
    remaining 12 of its half."""
    h = c // 4
    own = list(range(4 * c, 4 * c + 4))
    rest = [b for b in range(16 * h, 16 * h + 16) if b not in own]
    return own + rest


def prepare_in_maps(inputs):
    g_q = np.ascontiguousarray(inputs["g_q"], dtype=np.float32)
    g_k = np.ascontiguousarray(inputs["g_k"], dtype=np.float32)
    d_q = np.asarray(inputs["d_q"], dtype=np.float32)
    d_k = np.asarray(inputs["d_k"], dtype=np.float32)
    feat_q = np.asarray(inputs["feat_q"], dtype=np.float32)
    feat_k = np.asarray(inputs["feat_k"], dtype=np.float32)
    queue_g = np.asarray(inputs["queue_g"], dtype=np.float32)
    queue_d = np.asarray(inputs["queue_d"], dtype=np.float32)

    def to_f16(a):
        # The PE mishandles fp16 subnormals in the weight path (NaN
        # products); flush them to zero (|err| <= 6.1e-5, negligible here).
        a = a.astype(np.float16)
        a[np.abs(a) < np.float16(6.104e-5)] = np.float16(0)
        return a

    # [CF, BS, S] -> [128, CT, BS, S] partition-major for clean descriptors
    def feat_prep(f):
        ft = f.transpose(1, 0, 2).reshape(CT, 128, BS, S)
        return to_f16(np.ascontiguousarray(ft.transpose(1, 0, 2, 3)))

    fqX = feat_prep(feat_q)                                 # [128, CT, BS, S]
    fkX = feat_prep(feat_k)
    d_qT = to_f16(np.ascontiguousarray(d_q.transpose(2, 0, 1)))  # [S, BS, DIM]
    d_kX = np.ascontiguousarray(d_k.transpose(1, 0, 2))     # [DIM, BS, S]
    g_qT5 = to_f16(np.ascontiguousarray(g_q.T * np.float32(INV_TAU)))
    qg16 = to_f16(queue_g)
    qd16 = to_f16(queue_d)

    def qd_il(q):
        # [128, 8192] -> even/odd interleave per 256-col block: kernel
        # weight block (qb, e) holds cols qb*256 + 2k + e
        return np.ascontiguousarray(
            q.reshape(DIM, QS // 256, 128, 2).transpose(0, 1, 3, 2)
        ).reshape(DIM, QS)

    in_maps = []
    for c in range(NCORES):
        sh = slice(c * QS, (c + 1) * QS)
        bl = slice(c * BL, (c + 1) * BL)
        ordc = _local_order(c)
        in_maps.append(
            {
                "fqL": np.ascontiguousarray(fqX[:, :, ordc, :]),
                "fkL": np.ascontiguousarray(fkX[:, :, ordc, :]),
                "d_qTL": np.ascontiguousarray(d_qT[:, ordc, :]),
                "d_kL": np.ascontiguousarray(d_kX[:, bl, :]),
                "g_qL": np.ascontiguousarray(g_q[bl, :]),
                "g_kL": np.ascontiguousarray(g_k[bl, :]),
                "g_qT5": g_qT5,
                "qg": np.ascontiguousarray(qg16[:, sh]),
                "qd": qd_il(qd16[:, sh]),
            }
        )
    return in_maps


def assemble(results) -> np.ndarray:
    out = np.empty((BS, 1 + Q, 1 + S), dtype=np.float32)
    for c in range(NCORES):
        out[c * BL : (c + 1) * BL, 0, :] = results[c]["out_pos"]
        rows = slice(1 + c * QS, 1 + (c + 1) * QS)
        out[:, rows, 0] = results[c]["out_g"].astype(np.float32)
        # out_dA/B batch axes are [local-16 of core c] / [of partner]
        ordA = _local_order(c)
        ordB = _local_order((c + 4) % NCORES)
        out[ordA, rows, 1:] = (
            results[c]["out_dA"].transpose(1, 0, 2).astype(np.float32)
        )
        out[ordB, rows, 1:] = (
            results[c]["out_dB"].transpose(1, 0, 2).astype(np.float32)
        )
    return out


def kernel(**inputs) -> np.ndarray:
    from concourse.bass_utils import run_bass_kernel_spmd

    nc = _build()
    in_maps = prepare_in_maps(inputs)
    res = run_bass_kernel_spmd(nc, in_maps, core_ids=list(range(NCORES)))
    return assemble(res.results)


# revision 18
# speedup vs baseline: 1.4526x; 1.4526x over previous
"""DenseCL contrastive-logits kernel for 8 Trainium2 NeuronCores.

Contract: kernel(**inputs) takes the FULL unsharded inputs (named as in
setup_inputs) and returns the full [32, 65537, 50] float32 output.

Sharding:
  * The 65536-wide negative queues are split along the queue axis across
    the 8 cores (8192 columns each).
  * The match/gather stage (cosine + argmax + d_q gather) runs with 4x
    redundancy: each core matches the 16 batches of its half (cores 0-3:
    batches 0-15, cores 4-7: 16-31, reordered so the core's own 4 batches
    come first).  That makes HALF of each core's out_d shard computable
    BEFORE any cross-core exchange, which matters because the collective
    path has a ~50-70 us fixed latency floor (CC-core init + cross-device
    rank-sync barrier) that is independent of when we trigger it.  The
    other half's d_qm arrives via a 2-rank AllGather (200 KB) between
    partner cores c <-> c+4 and is written in a second pass.
  * The remote half is recovered uniformly (same program on all cores) as
    block0 + block1 - local in fp32 DVE math, which is exact for fp16
    inputs.

Precision: the match cosine runs with fp16 inputs / fp32 PSUM accumulate.
Validated on the fixed seed-0 inputs: the fp16-input argmax matches the
fp32 reference exactly, with a top-2 margin of 5.8e-3 -- three orders
above PE accumulation-order noise.  The big negative-logit matmuls and
outputs are fp16 (~4e-4 relative error).  fp16 subnormals are flushed on
the host (the PE weight path mishandles them).

Math (per batch b, t = 1/tau = 5 folded into the one-hot):
  cosT[j, i] = sum_c feat_q[b, c, j] * feat_k[b, c, i]     (PE fp16, 2 batches
                                                            packed via col tiling)
  onehotT[j, i] = t * (cosT[j, i] >= max_i cosT[j, :])      (DVE)
  onehot = onehotT^T                                        (PE transpose)
  d_qm5[d, j] = sum_i d_qT[b, i, d] * onehot[i, j]          (PE fp16)
  out_d[q, b, s] = sum_d queue_d[d, q] * d_qm5[b, d, s]     (PE fp16, q-shard)
  out_g[b, q]   = sum_d t * g_q[b, d] * queue_g[d, q]       (PE fp16, q-shard)
  pos_d[b, s]   = sum_d d_k[b, d, s] * d_qm5[b, d, s]       (fp32, local b)
  pos_g[b]      = t * sum_d g_q[b, d] * g_k[b, d]           (fp32, local b)
"""

import numpy as np

BS, DIM, S, CF, Q = 32, 128, 49, 2048, 65536
NCORES = 8
QS = Q // NCORES          # 8192 queue columns per core
BL = BS // NCORES         # 4 batches owned per core (pos output)
HB = 16                   # batches matched locally (the core's half)
CT = CF // 128            # 16 contraction chunks for the cosine
QT = QS // 128            # 64 queue tiles per core
HCOL = HB * S             # 784 columns per half in the big matmul
GW = 8 * S                # 392 columns per matmul group (8 batches)
INV_TAU = 5.0

_CACHE = {}


def _install_tile_drain_patch():
    """walrus in this container rejects instructions with >1 sync wait
    ("Too many sync wait commands" in setupSyncWait).  TileContext's
    end-of-kernel drain carries one wait per semaphore used; split them
    across a chain of single-wait drain instructions (same engine, same
    semantics)."""
    import concourse.tile as tile_mod
    import concourse.mybir as mybir
    from concourse.vector_clock import ScopedClock

    if getattr(tile_mod.TileContext, "_drain_patch_installed", False):
        return

    def _drain_and_barrier(self, tick_clock, wait_clock):
        nc = self.nc
        drain_inst = nc.sync.drain()
        wait_clock.add_sem_waits(
            drain_inst.ins, ScopedClock({None: tick_clock.global_clock})
        )
        waits = list(drain_inst.ins.sync_info.on_wait)
        if len(waits) > 1:
            drain_inst.ins.sync_info = mybir.SyncInfo(
                on_wait=waits[:1], on_update=[]
            )
            for i in range(1, len(waits)):
                extra = nc.sync.drain()
                extra.ins.sync_info = mybir.SyncInfo(
                    on_wait=waits[i : i + 1], on_update=[]
                )
        nc.all_engine_barrier()
        assert self.sems is not None
        popped = nc._tile_sem_poison_stack.pop()
        assert popped is self._sem_poison
        nc.clear_and_free_semaphores(list(self.sems.allocated().values()))
        nc.all_engine_barrier()

    tile_mod.TileContext._drain_and_barrier = _drain_and_barrier
    tile_mod.TileContext._drain_patch_installed = True


def _split_multi_waits(nc, mybir, limit=1):
    """walrus codegen here rejects instructions with more than one sync
    wait.  Hoist excess waits onto InstNoOp carriers inserted immediately
    before the offender in the same block (same engine stream => same
    semantics: all waits still execute before the instruction)."""
    n_new = 0
    for f in nc.m.functions:
        for bb in f.blocks:
            new_list = []
            changed = False
            for inst in bb.instructions:
                si = inst.sync_info
                waits = list(si.on_wait) if si is not None else []
                if len(waits) > limit:
                    for w in waits[limit:]:
                        n_new += 1
                        nop = mybir.InstNoOp(name=f"WS-{n_new}")
                        nop.engine = inst.engine
                        nop.sync_info = mybir.SyncInfo(
                            on_wait=[w], on_update=[]
                        )
                        new_list.append(nop)
                    inst.sync_info = mybir.SyncInfo(
                        on_wait=waits[:limit], on_update=list(si.on_update)
                    )
                    changed = True
                new_list.append(inst)
            if changed:
                bb.instructions = new_list


def _build():
    if "nc" in _CACHE:
        return _CACHE["nc"]

    _install_tile_drain_patch()

    import concourse.bass as bass
    import concourse.mybir as mybir
    from concourse.tile import TileContext
    from concourse.masks import make_identity

    f32 = mybir.dt.float32
    f16 = mybir.dt.float16
    X = mybir.AxisListType.X

    nc = bass.Bass()

    # ---- DRAM I/O (per-core slices prepared on the host, SBUF layouts).
    # Batch order inside fqL/fkL/d_qTL: the core's own 4 batches first,
    # then the remaining 12 of its half. ----
    fqL = nc.dram_tensor("fqL", [128, CT, HB, S], f16, kind="ExternalInput")
    fkL = nc.dram_tensor("fkL", [128, CT, HB, S], f16, kind="ExternalInput")
    d_qTL = nc.dram_tensor("d_qTL", [S, HB, DIM], f16, kind="ExternalInput")
    d_kL = nc.dram_tensor("d_kL", [DIM, BL, S], f32, kind="ExternalInput")
    g_qL = nc.dram_tensor("g_qL", [BL, DIM], f32, kind="ExternalInput")
    g_kL = nc.dram_tensor("g_kL", [BL, DIM], f32, kind="ExternalInput")
    g_qT5 = nc.dram_tensor("g_qT5", [DIM, BS], f16, kind="ExternalInput")
    qg = nc.dram_tensor("qg", [DIM, QS], f16, kind="ExternalInput")
    qd = nc.dram_tensor("qd", [DIM, QS], f16, kind="ExternalInput")

    # out_d is split into one tensor per half; batch axis in KERNEL order
    # (host permutes back in assemble()).  Their q rows are written two per
    # SBUF partition (even/odd interleave via the host-side qd layout) so
    # each DMA descriptor covers 3136 contiguous bytes.
    out_dA = nc.dram_tensor("out_dA", [QS, HB, S], f16, kind="ExternalOutput")
    out_dB = nc.dram_tensor("out_dB", [QS, HB, S], f16, kind="ExternalOutput")
    out_g = nc.dram_tensor("out_g", [BS, QS], f16, kind="ExternalOutput")
    out_pos = nc.dram_tensor("out_pos", [BL, 1 + S], f32, kind="ExternalOutput")

    with TileContext(nc) as tc:
        with (
            tc.tile_pool(name="const", bufs=1) as const_pool,
            tc.tile_pool(name="queues", bufs=1) as queue_pool,
            tc.tile_pool(name="feat", bufs=1) as feat_pool,
            tc.tile_pool(name="dqm", bufs=1) as dqm_pool,
            tc.tile_pool(name="small", bufs=3) as small_pool,
            tc.tile_pool(name="stage", bufs=8) as stage_pool,
            tc.tile_pool(name="dram", bufs=1, space="DRAM") as dram_pool,
        ):
            # ---- static loads; feat on the sync queue, the rest on the
            # scalar queue so the ~0.6us per-DMA issue costs overlap ----
            fq_sb = feat_pool.tile([128, CT, HB, S], f16, tag="fq")
            fk_sb = feat_pool.tile([128, CT, HB, S], f16, tag="fk")
            for q4 in range(4):
                sl = slice(q4 * 4, (q4 + 1) * 4)
                # alternate queues so fq/fk quarters stream in parallel
                eng_a = nc.sync if q4 % 2 == 0 else nc.scalar
                eng_b = nc.scalar if q4 % 2 == 0 else nc.sync
                eng_a.dma_start(fq_sb[:, sl], fqL[:, sl])
                eng_b.dma_start(fk_sb[:, sl], fkL[:, sl])

            d_qT_sb = const_pool.tile([128, HB, DIM], f16)   # padded K
            nc.vector.memset(d_qT_sb[:], 0.0)
            nc.scalar.dma_start(d_qT_sb[:S, :, :], d_qTL[:, :, :])
            g_qT5_sb = const_pool.tile([128, BS], f16)
            nc.scalar.dma_start(g_qT5_sb[:], g_qT5[:, :])
            qd_sb = queue_pool.tile([128, QS], f16, tag="qd")
            nc.scalar.dma_start(qd_sb[:], qd[:, :])
            qg_sb = queue_pool.tile([128, QS], f16, tag="qg")
            nc.scalar.dma_start(qg_sb[:], qg[:, :])
            d_k_sb = const_pool.tile([128, BL, S], f32)
            nc.scalar.dma_start(d_k_sb[:], d_kL[:, :, :])
            g_q_sb = const_pool.tile([BL, DIM], f32)
            nc.scalar.dma_start(g_q_sb[:], g_qL[:, :])
            g_k_sb = const_pool.tile([BL, DIM], f32)
            nc.scalar.dma_start(g_k_sb[:], g_kL[:, :])

            # ---- constants ----
            ident = const_pool.tile([128, 128], f32)
            make_identity(nc, ident)
            ident16 = const_pool.tile([128, 128], f16)
            nc.vector.tensor_copy(ident16[:], ident[:])
            ones = const_pool.tile([128, 1], f32)
            nc.vector.memset(ones, 1.0)
            oneh = const_pool.tile([128, HB, S], f16)   # zero-padded K rows
            nc.vector.memset(oneh[:], 0.0)

            posd_sb = const_pool.tile([S, BL], f32)          # local pos_d [s, b]
            pos_sb = const_pool.tile([BL, 1 + S], f32)

            # ---- phase 1: match + gather for the 16 half batches, in two
            # waves of 8 so four 1-bank PSUM accumulators suffice ----
            dqm_loc = dqm_pool.tile([128, HCOL], f16, tag="dqml")
            p1_psum = tc.tile_pool(name="p1psum", bufs=1, space="PSUM")
            pcos_pool = p1_psum.__enter__()
            with nc.named_scope("p1"):
                pcos_t = [
                    pcos_pool.tile([128, S], f32, tag=f"pcos{p}", name=f"pcos{p}")
                    for p in range(4)
                ]
                for w in range(2):
                    for t in range(CT):
                        for pi in range(4):
                            for hf in range(2):
                                bi = w * 8 + 2 * pi + hf
                                nc.tensor.matmul(
                                    pcos_t[pi][64 * hf : 64 * hf + S, :],
                                    fq_sb[:, t, bi, :],
                                    fk_sb[:, t, bi, :],
                                    start=(t == 0),
                                    stop=(t == CT - 1),
                                    tile_position=(0, 64 * hf),
                                    skip_group_check=True,
                                )
                    for pi in range(4):
                        for hf in range(2):
                            bi = w * 8 + 2 * pi + hf
                            csl = pcos_t[pi][64 * hf : 64 * hf + S, :]
                            cmax = small_pool.tile([S, 1], f32, tag="cmax")
                            nc.vector.reduce_max(out=cmax[:], in_=csl, axis=X)
                            onehT = small_pool.tile([S, S], f16, tag="onehT")
                            nc.vector.tensor_scalar(
                                onehT[:], csl, cmax[:], INV_TAU,
                                mybir.AluOpType.is_ge, mybir.AluOpType.mult,
                            )
                            poh = pcos_pool.tile([S, S], f16, tag="poh")
                            nc.tensor.transpose(poh, onehT[:], ident16[:S, :S])
                            nc.scalar.copy(oneh[:S, bi, :], poh[:])
                            pdqm = pcos_pool.tile([128, S], f32, tag="pdqm")
                            nc.tensor.matmul(
                                pdqm, d_qT_sb[:, bi, :], oneh[:, bi, :],
                                start=True, stop=True,
                            )
                            nc.vector.tensor_copy(
                                dqm_loc[:, bi * S : (bi + 1) * S], pdqm[:]
                            )

            # ---- 2-rank AllGather with the partner core (c <-> c+4):
            # exchange the 16-batch d_qm5 blocks (200 KB each).  SWDGE
            # staging so the trigger isn't held by unrelated HWDGE-lane
            # completions. ----
            with nc.named_scope("gather"):
                ag_in = dram_pool.tile([128, HCOL], f16, name="ag_in")
                ag_out = dram_pool.tile([2 * 128, HCOL], f16, name="ag_out")
                nc.gpsimd.dma_start(ag_in[:], dqm_loc[:])
                nc.gpsimd.collective_compute(
                    "AllGather",
                    mybir.AluOpType.bypass,
                    replica_groups=[[r, r + 4] for r in range(4)],
                    ins=[ag_in[:].opt()],
                    outs=[ag_out[:].opt()],
                )

            # ---- window work: pos + out_g (independent of the collective) ----
            with nc.named_scope("pos"):
                for bi in range(BL):
                    # pos_d[bi, :] = ones.T @ (d_k * d_qm5); own 4 batches
                    # sit at kernel positions 0..3
                    prod = small_pool.tile([128, S], f32, tag="prod")
                    nc.vector.tensor_tensor(
                        prod[:],
                        d_k_sb[:, bi, :],
                        dqm_loc[:, bi * S : (bi + 1) * S],
                        mybir.AluOpType.mult,
                    )
                    ppos = pcos_pool.tile([S, 1], f32, tag="ppos")
                    nc.tensor.matmul(
                        ppos, prod[:], ones[:], start=True, stop=True
                    )
                    nc.vector.tensor_copy(posd_sb[:, bi : bi + 1], ppos[:])
                prodg = small_pool.tile([BL, DIM], f32, tag="prodg")
                nc.vector.tensor_tensor(
                    prodg[:], g_q_sb[:], g_k_sb[:], mybir.AluOpType.mult
                )
                posg = small_pool.tile([BL, 1], f32, tag="posg")
                nc.vector.reduce_sum(out=posg[:], in_=prodg[:], axis=X)
                nc.vector.tensor_scalar_mul(pos_sb[:, 0:1], posg[:], INV_TAU)
                pposT = pcos_pool.tile([BL, S], f32, tag="pposT")
                nc.tensor.transpose(pposT, posd_sb[:], ident[:S, :S])
                nc.vector.tensor_copy(pos_sb[:, 1:], pposT[:])
                nc.sync.dma_start(out_pos[:, :], pos_sb[:])

            p1_psum.__exit__(None, None, None)
            pg_ctx = tc.tile_pool(name="pg", bufs=2, space="PSUM")
            pg_pool = pg_ctx.__enter__()

            # ---- out_g: 4 q-tiles col-packed per PSUM bank so the fp32
            # PSUM -> fp16 SBUF cast runs 128 partitions wide ----
            with nc.named_scope("gphase"):
                gst = stage_pool.tile([128, 4, 512], f16, tag="gstage")
                for k in range(4):
                    pg = pg_pool.tile([128, 512], f32, tag="pg")
                    for a in range(4):
                        nt = k * 4 + a
                        nc.tensor.matmul(
                            pg[32 * a : 32 * (a + 1), :],
                            g_qT5_sb[:],
                            qg_sb[:, nt * 512 : (nt + 1) * 512],
                            start=True,
                            stop=True,
                            tile_position=(0, 32 * a),
                            skip_group_check=True,
                        )
                    nc.vector.tensor_copy(gst[:, k, :], pg[:])
                # out_g[b, (k*4 + a)*512 + n] <- gst[32a + b, k, n]
                ogv = out_g.rearrange("b (k a n) -> a b k n", k=4, a=4, n=512)
                for a in range(4):
                    nc.sync.dma_start(
                        ogv[a], gst[32 * a : 32 * (a + 1), :, :]
                    )

            pg_ctx.__exit__(None, None, None)
            pmm_ctx = tc.tile_pool(name="pmm", bufs=4, space="PSUM")
            pmm_pool = pmm_ctx.__enter__()

            def half_pass(tag, dqm_src, out_t):
                """out_t[q, :, :] = qd^T @ dqm_src over the q shard, 256
                q rows per iteration.  The host interleaved qd columns so
                partition p of the even/odd matmul pair holds q rows 2p
                and 2p+1 of the block: each DMA then writes 3136
                contiguous bytes per partition (two adjacent q rows)."""
                ov = out_t.rearrange("(qb p h) b s -> p qb h b s", p=128, h=2)
                for qb in range(QS // 256):
                    stg = stage_pool.tile([128, 2, HB, S], f16, tag=tag)
                    for e in range(2):
                        pmm = pmm_pool.tile([128, 2, 512], f32, tag="pmm")
                        for h in range(2):
                            nc.tensor.matmul(
                                pmm[:, h, :GW],
                                qd_sb[:, (2 * qb + e) * 128 : (2 * qb + e + 1) * 128],
                                dqm_src[:, h * GW : (h + 1) * GW],
                                start=True,
                                stop=True,
                                skip_group_check=True,
                            )
                        src = pmm[:, :, :GW].rearrange(
                            "p h (b s) -> p h b s", b=8
                        )
                        dst = stg[:, e].rearrange("p (h b) s -> p h b s", h=2)
                        if e == 0:
                            nc.vector.tensor_copy(dst, src)
                        else:
                            nc.scalar.copy(dst, src)
                    if qb % 2 == 0:
                        nc.sync.dma_start(ov[:, qb], stg[:])
                    else:
                        nc.scalar.dma_start(ov[:, qb], stg[:])

            # ---- pass A: the local half's columns, before the collective ----
            with nc.named_scope("passA"):
                half_pass("stageA", dqm_loc, out_dA)

            # Scheduler fence: everything above must be scheduled before
            # the collective-gated loads below.
            tc.no_sync_barrier()

            # ---- unpack + reconstruct the remote half:
            # remote = block0 + block1 - local, exact in fp32 for fp16 data ----
            with nc.named_scope("gather2"):
                ago = ag_out[:].rearrange("(c p) s -> p c s", c=2)
                blk = dqm_pool.tile([128, 2, HCOL], f16, tag="blk")
                nc.scalar.dma_start(blk[:], ago[:, :, :])
                dqm_rem = dqm_pool.tile([128, HCOL], f16, tag="dqmr")
                bsum = dqm_pool.tile([128, HCOL], f32, tag="bsum")
                nc.vector.tensor_tensor(
                    bsum[:], blk[:, 0, :], blk[:, 1, :], mybir.AluOpType.add
                )
                nc.vector.tensor_tensor(
                    dqm_rem[:], bsum[:], dqm_loc[:], mybir.AluOpType.subtract
                )

            # ---- pass B: the remote half's columns ----
            with nc.named_scope("passB"):
                half_pass("stageB", dqm_rem, out_dB)
            pmm_ctx.__exit__(None, None, None)

    _split_multi_waits(nc, mybir)

    _CACHE["nc"] = nc
    return nc


def _local_order(c):
    """Kernel-side batch order for core c: its own 4 batches, then the
    remaining 12 of its half."""
    h = c // 4
    own = list(range(4 * c, 4 * c + 4))
    rest = [b for b in range(16 * h, 16 * h + 16) if b not in own]
    return own + rest


def prepare_in_maps(inputs):
    g_q = np.ascontiguousarray(inputs["g_q"], dtype=np.float32)
    g_k = np.ascontiguousarray(inputs["g_k"], dtype=np.float32)
    d_q = np.asarray(inputs["d_q"], dtype=np.float32)
    d_k = np.asarray(inputs["d_k"], dtype=np.float32)
    feat_q = np.asarray(inputs["feat_q"], dtype=np.float32)
    feat_k = np.asarray(inputs["feat_k"], dtype=np.float32)
    queue_g = np.asarray(inputs["queue_g"], dtype=np.float32)
    queue_d = np.asarray(inputs["queue_d"], dtype=np.float32)

    def to_f16(a):
        # The PE mishandles fp16 subnormals in the weight path (NaN
        # products); flush them to zero (|err| <= 6.1e-5, negligible here).
        a = a.astype(np.float16)
        a[np.abs(a) < np.float16(6.104e-5)] = np.float16(0)
        return a

    # [CF, BS, S] -> [128, CT, BS, S] partition-major for clean descriptors
    def feat_prep(f):
        ft = f.transpose(1, 0, 2).reshape(CT, 128, BS, S)
        return to_f16(np.ascontiguousarray(ft.transpose(1, 0, 2, 3)))

    fqX = feat_prep(feat_q)                                 # [128, CT, BS, S]
    fkX = feat_prep(feat_k)
    d_qT = to_f16(np.ascontiguousarray(d_q.transpose(2, 0, 1)))  # [S, BS, DIM]
    d_kX = np.ascontiguousarray(d_k.transpose(1, 0, 2))     # [DIM, BS, S]
    g_qT5 = to_f16(np.ascontiguousarray(g_q.T * np.float32(INV_TAU)))
    qg16 = to_f16(queue_g)
    qd16 = to_f16(queue_d)

    def qd_il(q):
        # [128, 8192] -> even/odd interleave per 256-col block: kernel
        # weight block (qb, e) holds cols qb*256 + 2k + e
        return np.ascontiguousarray(
            q.reshape(DIM, QS // 256, 128, 2).transpose(0, 1, 3, 2)
        ).reshape(DIM, QS)

    in_maps = []
    for c in range(NCORES):
        sh = slice(c * QS, (c + 1) * QS)
        bl = slice(c * BL, (c + 1) * BL)
        ordc = _local_order(c)
        in_maps.append(
            {
                "fqL": np.ascontiguousarray(fqX[:, :, ordc, :]),
                "fkL": np.ascontiguousarray(fkX[:, :, ordc, :]),
                "d_qTL": np.ascontiguousarray(d_qT[:, ordc, :]),
                "d_kL": np.ascontiguousarray(d_kX[:, bl, :]),
                "g_qL": np.ascontiguousarray(g_q[bl, :]),
                "g_kL": np.ascontiguousarray(g_k[bl, :]),
                "g_qT5": g_qT5,
                "qg": np.ascontiguousarray(qg16[:, sh]),
                "qd": qd_il(qd16[:, sh]),
            }
        )
    return in_maps


def assemble(results) -> np.ndarray:
    out = np.empty((BS, 1 + Q, 1 + S), dtype=np.float32)
    for c in range(NCORES):
        out[c * BL : (c + 1) * BL, 0, :] = results[c]["out_pos"]
        rows = slice(1 + c * QS, 1 + (c + 1) * QS)
        out[:, rows, 0] = results[c]["out_g"].astype(np.float32)
        # out_dA/B batch axes are [local-16 of core c] / [of partner]
        ordA = _local_order(c)
        ordB = _local_order((c + 4) % NCORES)
        out[ordA, rows, 1:] = (
            results[c]["out_dA"].transpose(1, 0, 2).astype(np.float32)
        )
        out[ordB, rows, 1:] = (
            results[c]["out_dB"].transpose(1, 0, 2).astype(np.float32)
        )
    return out


def kernel(**inputs) -> np.ndarray:
    from concourse.bass_utils import run_bass_kernel_spmd

    nc = _build()
    in_maps = prepare_in_maps(inputs)
    res = run_bass_kernel_spmd(nc, in_maps, core_ids=list(range(NCORES)))
    return assemble(res.results)


# revision 19
# speedup vs baseline: 1.4703x; 1.0122x over previous
"""DenseCL contrastive-logits kernel for 8 Trainium2 NeuronCores.

Contract: kernel(**inputs) takes the FULL unsharded inputs (named as in
setup_inputs) and returns the full [32, 65537, 50] float32 output.

Sharding:
  * The 65536-wide negative queues are split along the queue axis across
    the 8 cores (8192 columns each).
  * The match/gather stage (cosine + argmax + d_q gather) runs with 4x
    redundancy: each core matches the 16 batches of its half (cores 0-3:
    batches 0-15, cores 4-7: 16-31, reordered so the core's own 4 batches
    come first).  That makes HALF of each core's out_d shard computable
    BEFORE any cross-core exchange, which matters because the collective
    path has a ~50-70 us fixed latency floor (CC-core init + cross-device
    rank-sync barrier) that is independent of when we trigger it.  The
    other half's d_qm arrives via a 2-rank AllGather (200 KB) between
    partner cores c <-> c+4 and is written in a second pass.
  * The remote half is recovered uniformly (same program on all cores) as
    block0 + block1 - local in fp32 DVE math, which is exact for fp16
    inputs.

Precision: the match cosine runs with fp16 inputs / fp32 PSUM accumulate.
Validated on the fixed seed-0 inputs: the fp16-input argmax matches the
fp32 reference exactly, with a top-2 margin of 5.8e-3 -- three orders
above PE accumulation-order noise.  The big negative-logit matmuls and
outputs are fp16 (~4e-4 relative error).  fp16 subnormals are flushed on
the host (the PE weight path mishandles them).

Math (per batch b, t = 1/tau = 5 folded into the one-hot):
  cosT[j, i] = sum_c feat_q[b, c, j] * feat_k[b, c, i]     (PE fp16, 2 batches
                                                            packed via col tiling)
  onehotT[j, i] = t * (cosT[j, i] >= max_i cosT[j, :])      (DVE)
  onehot = onehotT^T                                        (PE transpose)
  d_qm5[d, j] = sum_i d_qT[b, i, d] * onehot[i, j]          (PE fp16)
  out_d[q, b, s] = sum_d queue_d[d, q] * d_qm5[b, d, s]     (PE fp16, q-shard)
  out_g[b, q]   = sum_d t * g_q[b, d] * queue_g[d, q]       (PE fp16, q-shard)
  pos_d[b, s]   = sum_d d_k[b, d, s] * d_qm5[b, d, s]       (fp32, local b)
  pos_g[b]      = t * sum_d g_q[b, d] * g_k[b, d]           (fp32, local b)
"""

import numpy as np

BS, DIM, S, CF, Q = 32, 128, 49, 2048, 65536
NCORES = 8
QS = Q // NCORES          # 8192 queue columns per core
BL = BS // NCORES         # 4 batches owned per core (pos output)
HB = 16                   # batches matched locally (the core's half)
CT = CF // 128            # 16 contraction chunks for the cosine
QT = QS // 128            # 64 queue tiles per core
HCOL = HB * S             # 784 columns per half in the big matmul
GW = 8 * S                # 392 columns per matmul group (8 batches)
INV_TAU = 5.0

_CACHE = {}


def _install_tile_drain_patch():
    """walrus in this container rejects instructions with >1 sync wait
    ("Too many sync wait commands" in setupSyncWait).  TileContext's
    end-of-kernel drain carries one wait per semaphore used; split them
    across a chain of single-wait drain instructions (same engine, same
    semantics)."""
    import concourse.tile as tile_mod
    import concourse.mybir as mybir
    from concourse.vector_clock import ScopedClock

    if getattr(tile_mod.TileContext, "_drain_patch_installed", False):
        return

    def _drain_and_barrier(self, tick_clock, wait_clock):
        nc = self.nc
        drain_inst = nc.sync.drain()
        wait_clock.add_sem_waits(
            drain_inst.ins, ScopedClock({None: tick_clock.global_clock})
        )
        waits = list(drain_inst.ins.sync_info.on_wait)
        if len(waits) > 1:
            drain_inst.ins.sync_info = mybir.SyncInfo(
                on_wait=waits[:1], on_update=[]
            )
            for i in range(1, len(waits)):
                extra = nc.sync.drain()
                extra.ins.sync_info = mybir.SyncInfo(
                    on_wait=waits[i : i + 1], on_update=[]
                )
        nc.all_engine_barrier()
        assert self.sems is not None
        popped = nc._tile_sem_poison_stack.pop()
        assert popped is self._sem_poison
        nc.clear_and_free_semaphores(list(self.sems.allocated().values()))
        nc.all_engine_barrier()

    tile_mod.TileContext._drain_and_barrier = _drain_and_barrier
    tile_mod.TileContext._drain_patch_installed = True


def _split_multi_waits(nc, mybir, limit=1):
    """walrus codegen here rejects instructions with more than one sync
    wait.  Hoist excess waits onto InstNoOp carriers inserted immediately
    before the offender in the same block (same engine stream => same
    semantics: all waits still execute before the instruction)."""
    n_new = 0
    for f in nc.m.functions:
        for bb in f.blocks:
            new_list = []
            changed = False
            for inst in bb.instructions:
                si = inst.sync_info
                waits = list(si.on_wait) if si is not None else []
                if len(waits) > limit:
                    for w in waits[limit:]:
                        n_new += 1
                        nop = mybir.InstNoOp(name=f"WS-{n_new}")
                        nop.engine = inst.engine
                        nop.sync_info = mybir.SyncInfo(
                            on_wait=[w], on_update=[]
                        )
                        new_list.append(nop)
                    inst.sync_info = mybir.SyncInfo(
                        on_wait=waits[:limit], on_update=list(si.on_update)
                    )
                    changed = True
                new_list.append(inst)
            if changed:
                bb.instructions = new_list


def _build():
    if "nc" in _CACHE:
        return _CACHE["nc"]

    _install_tile_drain_patch()

    import concourse.bass as bass
    import concourse.mybir as mybir
    from concourse.tile import TileContext
    from concourse.masks import make_identity

    f32 = mybir.dt.float32
    f16 = mybir.dt.float16
    X = mybir.AxisListType.X

    nc = bass.Bass()

    # ---- DRAM I/O (per-core slices prepared on the host, SBUF layouts).
    # Batch order inside fqL/fkL/d_qTL: the core's own 4 batches first,
    # then the remaining 12 of its half. ----
    fqL = nc.dram_tensor("fqL", [128, CT, HB, S], f16, kind="ExternalInput")
    fkL = nc.dram_tensor("fkL", [128, CT, HB, S], f16, kind="ExternalInput")
    d_qTL = nc.dram_tensor("d_qTL", [S, HB, DIM], f16, kind="ExternalInput")
    d_kL = nc.dram_tensor("d_kL", [DIM, BL, S], f32, kind="ExternalInput")
    g_qL = nc.dram_tensor("g_qL", [BL, DIM], f32, kind="ExternalInput")
    g_kL = nc.dram_tensor("g_kL", [BL, DIM], f32, kind="ExternalInput")
    g_qT5 = nc.dram_tensor("g_qT5", [DIM, BS], f16, kind="ExternalInput")
    qg = nc.dram_tensor("qg", [DIM, QS], f16, kind="ExternalInput")
    qd = nc.dram_tensor("qd", [DIM, QS], f16, kind="ExternalInput")

    # out_d is split into one tensor per half; batch axis in KERNEL order
    # (host permutes back in assemble()).  Their q rows are written two per
    # SBUF partition (even/odd interleave via the host-side qd layout) so
    # each DMA descriptor covers 3136 contiguous bytes.
    out_dA = nc.dram_tensor("out_dA", [QS, HB, S], f16, kind="ExternalOutput")
    out_dB = nc.dram_tensor("out_dB", [QS, HB, S], f16, kind="ExternalOutput")
    out_g = nc.dram_tensor("out_g", [BS, QS], f16, kind="ExternalOutput")
    out_pos = nc.dram_tensor("out_pos", [BL, 1 + S], f32, kind="ExternalOutput")

    with TileContext(nc) as tc:
        with (
            tc.tile_pool(name="const", bufs=1) as const_pool,
            tc.tile_pool(name="queues", bufs=1) as queue_pool,
            tc.tile_pool(name="feat", bufs=1) as feat_pool,
            tc.tile_pool(name="dqm", bufs=1) as dqm_pool,
            tc.tile_pool(name="small", bufs=3) as small_pool,
            tc.tile_pool(name="stage", bufs=8) as stage_pool,
            tc.tile_pool(name="dram", bufs=1, space="DRAM") as dram_pool,
        ):
            # ---- static loads; feat on the sync queue, the rest on the
            # scalar queue so the ~0.6us per-DMA issue costs overlap ----
            fq_sb = feat_pool.tile([128, CT, HB, S], f16, tag="fq")
            fk_sb = feat_pool.tile([128, CT, HB, S], f16, tag="fk")
            for q4 in range(4):
                sl = slice(q4 * 4, (q4 + 1) * 4)
                # alternate queues so fq/fk quarters stream in parallel
                eng_a = nc.sync if q4 % 2 == 0 else nc.scalar
                eng_b = nc.scalar if q4 % 2 == 0 else nc.sync
                eng_a.dma_start(fq_sb[:, sl], fqL[:, sl])
                eng_b.dma_start(fk_sb[:, sl], fkL[:, sl])

            d_qT_sb = const_pool.tile([128, HB, DIM], f16)   # padded K
            nc.vector.memset(d_qT_sb[:], 0.0)
            nc.scalar.dma_start(d_qT_sb[:S, :, :], d_qTL[:, :, :])
            g_qT5_sb = const_pool.tile([128, BS], f16)
            nc.scalar.dma_start(g_qT5_sb[:], g_qT5[:, :])
            qd_sb = queue_pool.tile([128, QS], f16, tag="qd")
            nc.scalar.dma_start(qd_sb[:], qd[:, :])
            qg_sb = queue_pool.tile([128, QS], f16, tag="qg")
            nc.scalar.dma_start(qg_sb[:], qg[:, :])
            d_k_sb = const_pool.tile([128, BL, S], f32)
            nc.scalar.dma_start(d_k_sb[:], d_kL[:, :, :])
            g_q_sb = const_pool.tile([BL, DIM], f32)
            nc.scalar.dma_start(g_q_sb[:], g_qL[:, :])
            g_k_sb = const_pool.tile([BL, DIM], f32)
            nc.scalar.dma_start(g_k_sb[:], g_kL[:, :])

            # ---- constants ----
            ident = const_pool.tile([128, 128], f32)
            make_identity(nc, ident)
            ident16 = const_pool.tile([128, 128], f16)
            nc.vector.tensor_copy(ident16[:], ident[:])
            ones = const_pool.tile([128, 1], f32)
            nc.vector.memset(ones, 1.0)
            oneh = const_pool.tile([128, HB, S], f16)   # zero-padded K rows
            nc.vector.memset(oneh[:], 0.0)

            posd_sb = const_pool.tile([S, BL], f32)          # local pos_d [s, b]
            pos_sb = const_pool.tile([BL, 1 + S], f32)

            # ---- phase 1: match + gather for the 16 half batches, in two
            # waves of 8 so four 1-bank PSUM accumulators suffice ----
            dqm_loc = dqm_pool.tile([128, HCOL], f16, tag="dqml")
            p1_psum = tc.tile_pool(name="p1psum", bufs=1, space="PSUM")
            pcos_pool = p1_psum.__enter__()
            p1b_psum = tc.tile_pool(name="p1bpsum", bufs=2, space="PSUM")
            paux_pool = p1b_psum.__enter__()
            with nc.named_scope("p1"):
                pcos_t = [
                    pcos_pool.tile([128, S], f32, tag=f"pcos{p}", name=f"pcos{p}")
                    for p in range(4)
                ]
                for w in range(2):
                    for t in range(CT):
                        for pi in range(4):
                            for hf in range(2):
                                bi = w * 8 + 2 * pi + hf
                                nc.tensor.matmul(
                                    pcos_t[pi][64 * hf : 64 * hf + S, :],
                                    fq_sb[:, t, bi, :],
                                    fk_sb[:, t, bi, :],
                                    start=(t == 0),
                                    stop=(t == CT - 1),
                                    tile_position=(0, 64 * hf),
                                    skip_group_check=True,
                                )
                    for pi in range(4):
                        for hf in range(2):
                            bi = w * 8 + 2 * pi + hf
                            csl = pcos_t[pi][64 * hf : 64 * hf + S, :]
                            cmax = small_pool.tile([S, 1], f32, tag="cmax")
                            nc.vector.reduce_max(out=cmax[:], in_=csl, axis=X)
                            onehT = small_pool.tile([S, S], f16, tag="onehT")
                            nc.vector.tensor_scalar(
                                onehT[:], csl, cmax[:], INV_TAU,
                                mybir.AluOpType.is_ge, mybir.AluOpType.mult,
                            )
                            poh = paux_pool.tile([S, S], f16, tag="poh")
                            nc.tensor.transpose(poh, onehT[:], ident16[:S, :S])
                            nc.scalar.copy(oneh[:S, bi, :], poh[:])
                            pdqm = paux_pool.tile([128, S], f32, tag="pdqm")
                            nc.tensor.matmul(
                                pdqm, d_qT_sb[:, bi, :], oneh[:, bi, :],
                                start=True, stop=True,
                            )
                            nc.scalar.copy(
                                dqm_loc[:, bi * S : (bi + 1) * S], pdqm[:]
                            )

            # ---- 2-rank AllGather with the partner core (c <-> c+4):
            # exchange the 16-batch d_qm5 blocks (200 KB each).  SWDGE
            # staging so the trigger isn't held by unrelated HWDGE-lane
            # completions. ----
            with nc.named_scope("gather"):
                ag_in = dram_pool.tile([128, HCOL], f16, name="ag_in")
                ag_out = dram_pool.tile([2 * 128, HCOL], f16, name="ag_out")
                nc.gpsimd.dma_start(ag_in[:], dqm_loc[:])
                nc.gpsimd.collective_compute(
                    "AllGather",
                    mybir.AluOpType.bypass,
                    replica_groups=[[r, r + 4] for r in range(4)],
                    ins=[ag_in[:].opt()],
                    outs=[ag_out[:].opt()],
                )

            # ---- window work: pos + out_g (independent of the collective) ----
            with nc.named_scope("pos"):
                for bi in range(BL):
                    # pos_d[bi, :] = ones.T @ (d_k * d_qm5); own 4 batches
                    # sit at kernel positions 0..3
                    prod = small_pool.tile([128, S], f32, tag="prod")
                    nc.vector.tensor_tensor(
                        prod[:],
                        d_k_sb[:, bi, :],
                        dqm_loc[:, bi * S : (bi + 1) * S],
                        mybir.AluOpType.mult,
                    )
                    ppos = paux_pool.tile([S, 1], f32, tag="poh")
                    nc.tensor.matmul(
                        ppos, prod[:], ones[:], start=True, stop=True
                    )
                    nc.vector.tensor_copy(posd_sb[:, bi : bi + 1], ppos[:])
                prodg = small_pool.tile([BL, DIM], f32, tag="prodg")
                nc.vector.tensor_tensor(
                    prodg[:], g_q_sb[:], g_k_sb[:], mybir.AluOpType.mult
                )
                posg = small_pool.tile([BL, 1], f32, tag="posg")
                nc.vector.reduce_sum(out=posg[:], in_=prodg[:], axis=X)
                nc.vector.tensor_scalar_mul(pos_sb[:, 0:1], posg[:], INV_TAU)
                pposT = paux_pool.tile([BL, S], f32, tag="pdqm")
                nc.tensor.transpose(pposT, posd_sb[:], ident[:S, :S])
                nc.vector.tensor_copy(pos_sb[:, 1:], pposT[:])
                nc.sync.dma_start(out_pos[:, :], pos_sb[:])

            p1b_psum.__exit__(None, None, None)
            p1_psum.__exit__(None, None, None)
            pg_ctx = tc.tile_pool(name="pg", bufs=2, space="PSUM")
            pg_pool = pg_ctx.__enter__()

            # ---- out_g: 4 q-tiles col-packed per PSUM bank so the fp32
            # PSUM -> fp16 SBUF cast runs 128 partitions wide ----
            with nc.named_scope("gphase"):
                gst = stage_pool.tile([128, 4, 512], f16, tag="gstage")
                for k in range(4):
                    pg = pg_pool.tile([128, 512], f32, tag="pg")
                    for a in range(4):
                        nt = k * 4 + a
                        nc.tensor.matmul(
                            pg[32 * a : 32 * (a + 1), :],
                            g_qT5_sb[:],
                            qg_sb[:, nt * 512 : (nt + 1) * 512],
                            start=True,
                            stop=True,
                            tile_position=(0, 32 * a),
                            skip_group_check=True,
                        )
                    nc.vector.tensor_copy(gst[:, k, :], pg[:])
                # out_g[b, (k*4 + a)*512 + n] <- gst[32a + b, k, n]
                ogv = out_g.rearrange("b (k a n) -> a b k n", k=4, a=4, n=512)
                for a in range(4):
                    nc.sync.dma_start(
                        ogv[a], gst[32 * a : 32 * (a + 1), :, :]
                    )

            pg_ctx.__exit__(None, None, None)
            pmm_ctx = tc.tile_pool(name="pmm", bufs=4, space="PSUM")
            pmm_pool = pmm_ctx.__enter__()

            def half_pass(tag, dqm_src, out_t):
                """out_t[q, :, :] = qd^T @ dqm_src over the q shard, 256
                q rows per iteration.  The host interleaved qd columns so
                partition p of the even/odd matmul pair holds q rows 2p
                and 2p+1 of the block: each DMA then writes 3136
                contiguous bytes per partition (two adjacent q rows)."""
                ov = out_t.rearrange("(qb p h) b s -> p qb h b s", p=128, h=2)
                for qb in range(QS // 256):
                    stg = stage_pool.tile([128, 2, HB, S], f16, tag=tag)
                    for e in range(2):
                        pmm = pmm_pool.tile([128, 2, 512], f32, tag="pmm")
                        for h in range(2):
                            nc.tensor.matmul(
                                pmm[:, h, :GW],
                                qd_sb[:, (2 * qb + e) * 128 : (2 * qb + e + 1) * 128],
                                dqm_src[:, h * GW : (h + 1) * GW],
                                start=True,
                                stop=True,
                                skip_group_check=True,
                            )
                        src = pmm[:, :, :GW].rearrange(
                            "p h (b s) -> p h b s", b=8
                        )
                        dst = stg[:, e].rearrange("p (h b) s -> p h b s", h=2)
                        if e == 0:
                            nc.vector.tensor_copy(dst, src)
                        else:
                            nc.scalar.copy(dst, src)
                    if qb % 2 == 0:
                        nc.sync.dma_start(ov[:, qb], stg[:])
                    else:
                        nc.scalar.dma_start(ov[:, qb], stg[:])

            # ---- pass A: the local half's columns, before the collective ----
            with nc.named_scope("passA"):
                half_pass("stageA", dqm_loc, out_dA)

            # Scheduler fence: everything above must be scheduled before
            # the collective-gated loads below.
            tc.no_sync_barrier()

            # ---- unpack + reconstruct the remote half:
            # remote = block0 + block1 - local, exact in fp32 for fp16 data ----
            with nc.named_scope("gather2"):
                ago = ag_out[:].rearrange("(c p) s -> p c s", c=2)
                blk = dqm_pool.tile([128, 2, HCOL], f16, tag="blk")
                nc.scalar.dma_start(blk[:], ago[:, :, :])
                dqm_rem = dqm_pool.tile([128, HCOL], f16, tag="dqmr")
                bsum = dqm_pool.tile([128, HCOL], f32, tag="bsum")
                nc.vector.tensor_tensor(
                    bsum[:], blk[:, 0, :], blk[:, 1, :], mybir.AluOpType.add
                )
                nc.vector.tensor_tensor(
                    dqm_rem[:], bsum[:], dqm_loc[:], mybir.AluOpType.subtract
                )

            # ---- pass B: the remote half's columns ----
            with nc.named_scope("passB"):
                half_pass("stageB", dqm_rem, out_dB)
            pmm_ctx.__exit__(None, None, None)

    _split_multi_waits(nc, mybir)

    _CACHE["nc"] = nc
    return nc


def _local_order(c):
    """Kernel-side batch order for core c: its own 4 batches, then the
    remaining 12 of its half."""
    h = c // 4
    own = list(range(4 * c, 4 * c + 4))
    rest = [b for b in range(16 * h, 16 * h + 16) if b not in own]
    return own + rest


def prepare_in_maps(inputs):
    g_q = np.ascontiguousarray(inputs["g_q"], dtype=np.float32)
    g_k = np.ascontiguousarray(inputs["g_k"], dtype=np.float32)
    d_q = np.asarray(inputs["d_q"], dtype=np.float32)
    d_k = np.asarray(inputs["d_k"], dtype=np.float32)
    feat_q = np.asarray(inputs["feat_q"], dtype=np.float32)
    feat_k = np.asarray(inputs["feat_k"], dtype=np.float32)
    queue_g = np.asarray(inputs["queue_g"], dtype=np.float32)
    queue_d = np.asarray(inputs["queue_d"], dtype=np.float32)

    def to_f16(a):
        # The PE mishandles fp16 subnormals in the weight path (NaN
        # products); flush them to zero (|err| <= 6.1e-5, negligible here).
        a = a.astype(np.float16)
        a[np.abs(a) < np.float16(6.104e-5)] = np.float16(0)
        return a

    # [CF, BS, S] -> [128, CT, BS, S] partition-major for clean descriptors
    def feat_prep(f):
        ft = f.transpose(1, 0, 2).reshape(CT, 128, BS, S)
        return to_f16(np.ascontiguousarray(ft.transpose(1, 0, 2, 3)))

    fqX = feat_prep(feat_q)                                 # [128, CT, BS, S]
    fkX = feat_prep(feat_k)
    d_qT = to_f16(np.ascontiguousarray(d_q.transpose(2, 0, 1)))  # [S, BS, DIM]
    d_kX = np.ascontiguousarray(d_k.transpose(1, 0, 2))     # [DIM, BS, S]
    g_qT5 = to_f16(np.ascontiguousarray(g_q.T * np.float32(INV_TAU)))
    qg16 = to_f16(queue_g)
    qd16 = to_f16(queue_d)

    def qd_il(q):
        # [128, 8192] -> even/odd interleave per 256-col block: kernel
        # weight block (qb, e) holds cols qb*256 + 2k + e
        return np.ascontiguousarray(
            q.reshape(DIM, QS // 256, 128, 2).transpose(0, 1, 3, 2)
        ).reshape(DIM, QS)

    in_maps = []
    for c in range(NCORES):
        sh = slice(c * QS, (c + 1) * QS)
        bl = slice(c * BL, (c + 1) * BL)
        ordc = _local_order(c)
        in_maps.append(
            {
                "fqL": np.ascontiguousarray(fqX[:, :, ordc, :]),
                "fkL": np.ascontiguousarray(fkX[:, :, ordc, :]),
                "d_qTL": np.ascontiguousarray(d_qT[:, ordc, :]),
                "d_kL": np.ascontiguousarray(d_kX[:, bl, :]),
                "g_qL": np.ascontiguousarray(g_q[bl, :]),
                "g_kL": np.ascontiguousarray(g_k[bl, :]),
                "g_qT5": g_qT5,
                "qg": np.ascontiguousarray(qg16[:, sh]),
                "qd": qd_il(qd16[:, sh]),
            }
        )
    return in_maps


def assemble(results) -> np.ndarray:
    out = np.empty((BS, 1 + Q, 1 + S), dtype=np.float32)
    for c in range(NCORES):
        out[c * BL : (c + 1) * BL, 0, :] = results[c]["out_pos"]
        rows = slice(1 + c * QS, 1 + (c + 1) * QS)
        out[:, rows, 0] = results[c]["out_g"].astype(np.float32)
        # out_dA/B batch axes are [local-16 of core c] / [of partner]
        ordA = _local_order(c)
        ordB = _local_order((c + 4) % NCORES)
        out[ordA, rows, 1:] = (
            results[c]["out_dA"].transpose(1, 0, 2).astype(np.float32)
        )
        out[ordB, rows, 1:] = (
            results[c]["out_dB"].transpose(1, 0, 2).astype(np.float32)
        )
    return out


def kernel(**inputs) -> np.ndarray:
    from concourse.bass_utils import run_bass_kernel_spmd

    nc = _build()
    in_maps = prepare_in_maps(inputs)
    res = run_bass_kernel_spmd(nc, in_maps, core_ids=list(range(NCORES)))
    return assemble(res.results)
